# revision 1
# baseline (speedup 1.0000x reference)
"""Trainium2 Bass kernel for the hierarchical GNN (GINConv x2 + community pooling).

Math notes (vs the PyTorch/JAX reference):
  - softmax(alpha, axis=1) of an (E,1) tensor is identically 1, so the
    attention path collapses: conv(x) = segment_sum(xp[col], row) with
    xp = x @ W.  edge_attr / We / Wa are dead.
  - cluster0[i] == i//10, cluster1[i] == i//10, batch2[i] == i//40 (verified
    at runtime): the segment_max pools are max over blocks of 10 consecutive
    rows, and the per-graph mean divides by exactly 40.

Per-core layout (8 graphs per NeuronCore, data parallel over 8 cores):
  - conv1 runs in transposed layout: a [128, 4096] SBUF table holds xp^T for
    8 graphs (16 features x 8 graph-groups of 16 partitions).  Edge messages
    are gathered with GPSIMD ap_gather (per-16-partition-group indices = one
    graph per group), summed per destination via a chained DVE
    tensor_tensor_scan (prefix sum over dest-sorted edges) and a second
    ap_gather at segment boundaries + diff.
  - community pooling = strided max-reduce over blocks of 10 columns.
  - conv2 uses a dense per-graph 400x400 pooled adjacency (host-built from
    edge_index1) on the TensorEngine.
"""

import sys

sys.path.insert(0, "/opt/trn_rl_repo")

import numpy as np

B = 64
NPG = 4000
CPG0 = 400
CPG1 = 40
DEG = 8
F_IN = 64
F1 = 16
F2 = 32
EPG = NPG * DEG          # 32000 edges per graph
NCORES = 8
GPC = B // NCORES        # 8 graphs per core
NPC = GPC * NPG          # 32000 nodes per core
HALF = EPG // 2          # 16000 edges per scan half
CHUNK = 2000             # edges per gather/scan chunk
NCH = HALF // CHUNK      # 8 chunks per half
TBL = 4096               # xp^T table width (>= NPG)
BPAD = 4016              # boundary gather count (4001 rounded to x16)
XTC = 2000               # x^T DMA chunk columns
PJ = 500                 # projection matmul free-dim


def _grp(g):
    """Local graph id -> 16-partition table group (pairing (p, p+4))."""
    return 2 * g if g < 4 else 2 * (g - 4) + 1


_PROGRAM = None
_PROGRAM_REPS = None
_DEBUG = False


def _build_program(reps=1):
    import concourse.bacc as bacc
    import concourse.mybir as mybir
    from concourse.tile import TileContext

    f32 = mybir.dt.float32
    i16 = mybir.dt.int16
    AF = mybir.ActivationFunctionType
    OP = mybir.AluOpType

    nc = bacc.Bacc(None, target_bir_lowering=False)

    xt_in = nc.declare_dram_parameter("xt", [128, NPC // 2], f32, isOutput=False)
    eidx_in = nc.declare_dram_parameter("eidx", [128, EPG // 16], i16, isOutput=False)
    bidx_in = nc.declare_dram_parameter("bidx", [128, 2 * BPAD // 16], i16, isOutput=False)
    a1t_in = nc.declare_dram_parameter("a1t", [100, GPC * 4 * CPG0], f32, isOutput=False)
    w1bd_in = nc.declare_dram_parameter("w1bd", [128, 2 * F1], f32, isOutput=False)
    w2big_in = nc.declare_dram_parameter("w2big", [128, 4 * F2], f32, isOutput=False)
    fc1big_in = nc.declare_dram_parameter("fc1big", [128, 128], f32, isOutput=False)
    fc1b_in = nc.declare_dram_parameter("fc1b", [64, 1], f32, isOutput=False)
    fc2w_in = nc.declare_dram_parameter("fc2w", [64, 1], f32, isOutput=False)
    fc2b_in = nc.declare_dram_parameter("fc2b", [1, 1], f32, isOutput=False)
    out_ext = nc.declare_dram_parameter("out", [1, GPC], f32, isOutput=True)
    dbg = {}
    if _DEBUG:
        dbg["xpT"] = nc.declare_dram_parameter("dbg_xpT", [128, TBL], f32, isOutput=True)
        dbg["P"] = nc.declare_dram_parameter("dbg_P", [128, HALF + 4], f32, isOutput=True)
        dbg["b0"] = nc.declare_dram_parameter("dbg_b0", [128, BPAD], f32, isOutput=True)
        dbg["b1"] = nc.declare_dram_parameter("dbg_b1", [128, BPAD], f32, isOutput=True)
        dbg["m1"] = nc.declare_dram_parameter("dbg_m1", [128, NPG], f32, isOutput=True)
        dbg["xpool"] = nc.declare_dram_parameter("dbg_xpool", [128, CPG0], f32, isOutput=True)
        dbg["x2T0"] = nc.declare_dram_parameter("dbg_x2T0", [128, CPG0], f32, isOutput=True)
        dbg["x2T1"] = nc.declare_dram_parameter("dbg_x2T1", [128, CPG0], f32, isOutput=True)

    with TileContext(nc) as tc:
        with (
            tc.tile_pool(name="sb", bufs=1) as sb,
            tc.tile_pool(name="xtp", bufs=2) as xtp,
            tc.tile_pool(name="gp", bufs=2) as gp,
            tc.tile_pool(name="ip", bufs=3) as ip,
            tc.tile_pool(name="a1p", bufs=2) as a1p,
            tc.tile_pool(name="ps", bufs=8, space="PSUM") as psp,
        ):
            w1bd = sb.tile([128, 2 * F1], f32)
            w2big = sb.tile([128, 4 * F2], f32)
            fc1big = sb.tile([128, 128], f32)
            fc1b = sb.tile([64, 1], f32)
            fc2w = sb.tile([64, 1], f32)
            fc2b = sb.tile([1, 1], f32)
            xpT = sb.tile([128, TBL], f32)
            P = sb.tile([128, HALF + 4], f32)
            b0 = sb.tile([128, BPAD], f32)
            b1 = sb.tile([128, BPAD], f32)
            m1 = sb.tile([128, NPG], f32)
            xpool = sb.tile([128, CPG0], f32)
            x2T = [sb.tile([128, CPG0], f32, name=f"x2T{s}") for s in range(2)]
            x3T = [sb.tile([128, CPG1], f32, name=f"x3T{s}") for s in range(2)]
            xg = [sb.tile([128, 1], f32, name=f"xg{s}") for s in range(2)]
            outt = sb.tile([1, GPC], f32)
            zero = sb.tile([128, 1], f32)

            for _rep in range(reps):
                nc.sync.dma_start(out=w1bd[:], in_=w1bd_in[:])
                nc.sync.dma_start(out=w2big[:], in_=w2big_in[:])
                nc.sync.dma_start(out=fc1big[:], in_=fc1big_in[:])
                nc.sync.dma_start(out=fc1b[:], in_=fc1b_in[:])
                nc.sync.dma_start(out=fc2w[:], in_=fc2w_in[:])
                nc.sync.dma_start(out=fc2b[:], in_=fc2b_in[:])
                nc.vector.memset(zero[:], 0.0)
                nc.vector.memset(P[:, 0:1], 0.0)

                # ---- stage 1: xp^T = (x @ W1)^T via paired block-diagonal W1 ----
                # xt chunk j covers x^T columns [XTC*j, XTC*(j+1)) of the packed
                # [128, 16000] layout; graph pair p = chunk column / NPG.
                for j in range(NPC // 2 // XTC):
                    xtt = xtp.tile([128, XTC], f32, name="xtt")
                    nc.sync.dma_start(out=xtt[:], in_=xt_in[:, XTC * j : XTC * (j + 1)])
                    p = (XTC * j) // NPG
                    for i in range(XTC // PJ):
                        col = XTC * j + PJ * i - NPG * p  # table column base
                        ps = psp.tile([F2, PJ], f32, name="pj", tag="ps")
                        nc.tensor.matmul(
                            ps[:], w1bd[:], xtt[:, PJ * i : PJ * (i + 1)],
                            start=True, stop=True,
                        )
                        nc.scalar.activation(
                            xpT[32 * p : 32 * p + 32, col : col + PJ], ps[:], AF.Copy
                        )

                # ---- stage 2: conv1 aggregation (gather + prefix scan + diff) ----
                zbc = zero[:, 0:1].broadcast_to((128, CHUNK))
                for hh, bt in ((0, b0), (1, b1)):
                    for k in range(NCH):
                        g = gp.tile([128, CHUNK], f32, name="gch")
                        ic = (HALF // 16) * hh + (CHUNK // 16) * k
                        # ap_gather ucode ignores the idx AP offset: stage each
                        # chunk's indices into an offset-0 tile via DMA.
                        et = ip.tile([128, CHUNK // 16], i16, name="et")
                        nc.sync.dma_start(out=et[:], in_=eidx_in[:, ic : ic + CHUNK // 16])
                        nc.gpsimd.ap_gather(
                            g[:], xpT[:], et[:],
                            channels=128, num_elems=TBL, d=1, num_idxs=CHUNK,
                        )
                        init = 0.0 if k == 0 else P[:, CHUNK * k : CHUNK * k + 1]
                        nc.vector.tensor_tensor_scan(
                            P[:, 1 + CHUNK * k : 1 + CHUNK * (k + 1)],
                            g[:], zbc, init, OP.add, OP.add,
                        )
                    bix = ip.tile([128, BPAD // 16], i16, name="bix", bufs=2)
                    nc.sync.dma_start(
                        out=bix[:],
                        in_=bidx_in[:, (BPAD // 16) * hh : (BPAD // 16) * (hh + 1)],
                    )
                    nc.gpsimd.ap_gather(
                        bt[:], P[:, 0 : HALF + 1], bix[:],
                        channels=128, num_elems=HALF + 1, d=1, num_idxs=BPAD,
                    )
                nc.vector.tensor_tensor(
                    b0[:, 0 : NPG + 1], b0[:, 0 : NPG + 1], b1[:, 0 : NPG + 1], OP.add
                )
                nc.vector.tensor_tensor(
                    m1[:], b0[:, 1 : NPG + 1], b0[:, 0:NPG], OP.subtract
                )
                # pool1: relu(max over blocks of 10) == max(relu) since relu monotone
                nc.vector.tensor_reduce(
                    xpool[:], m1[:].rearrange("p (a b) -> p a b", b=10),
                    mybir.AxisListType.X, OP.max,
                )
                nc.scalar.activation(xpool[:], xpool[:], AF.Relu)

                # ---- stage 3: conv2 = relu(A1 @ (x_pool @ W2)) per graph ----
                for g in range(GPC):
                    t = _grp(g)
                    q, u = t // 4, t % 4
                    a1 = a1p.tile([100, 4, CPG0], f32, name="a1g")
                    nc.sync.dma_start(
                        out=a1[:], in_=a1t_in[:, 4 * CPG0 * g : 4 * CPG0 * (g + 1)]
                    )
                    xp2t = sb.tile([100, 4, F2], f32, name="xp2t", bufs=2)
                    for ch in range(4):
                        ps = psp.tile([100, F2], f32, name="p2", tag="ps")
                        nc.tensor.matmul(
                            ps[:],
                            xpool[64 * q : 64 * (q + 1), 100 * ch : 100 * (ch + 1)],
                            w2big[64 * q : 64 * (q + 1), F2 * u : F2 * (u + 1)],
                            start=True, stop=True,
                        )
                        nc.scalar.activation(xp2t[:, ch, :], ps[:], AF.Copy)
                    psm = psp.tile([F2, CPG0], f32, name="pm", tag="ps")
                    for ch in range(4):
                        nc.tensor.matmul(
                            psm[:], xp2t[:, ch, :], a1[:, ch, :],
                            start=(ch == 0), stop=(ch == 3),
                        )
                    s, slot = g // 4, g % 4
                    nc.scalar.activation(
                        x2T[s][32 * slot : 32 * (slot + 1), :], psm[:], AF.Relu
                    )

                # ---- stage 4: pool2 + graph mean + MLP ----
                for s in range(2):
                    nc.vector.tensor_reduce(
                        x3T[s][:], x2T[s][:].rearrange("p (a b) -> p a b", b=10),
                        mybir.AxisListType.X, OP.max,
                    )
                    nc.vector.tensor_reduce(
                        xg[s][:], x3T[s][:], mybir.AxisListType.X, OP.add
                    )
                for g in range(GPC):
                    s, slot = g // 4, g % 4
                    q2, u2 = slot // 2, slot % 2
                    psh = psp.tile([64, 1], f32, name="ph", tag="ps")
                    nc.tensor.matmul(
                        psh[:],
                        fc1big[64 * q2 : 64 * (q2 + 1), 64 * u2 : 64 * (u2 + 1)],
                        xg[s][64 * q2 : 64 * (q2 + 1), 0:1],
                        start=True, stop=True,
                    )
                    ht = sb.tile([64, 1], f32, name="ht", bufs=2)
                    nc.scalar.activation(
                        ht[:], psh[:], AF.Relu, bias=fc1b[:], scale=1.0 / CPG1
                    )
                    pso = psp.tile([1, 1], f32, name="po", tag="ps")
                    nc.tensor.matmul(pso[:], fc2w[:], ht[:], start=True, stop=True)
                    # final linear has no relu: out = h @ fc2_W + fc2_b
                    nc.vector.scalar_tensor_tensor(
                        outt[0:1, g : g + 1], pso[:], 1.0, fc2b[0:1, 0:1],
                        OP.mult, OP.add,
                    )
                nc.sync.dma_start(out=out_ext[:], in_=outt[:])
            if _DEBUG:
                nc.sync.dma_start(out=dbg["xpT"][:], in_=xpT[:])
                nc.sync.dma_start(out=dbg["P"][:], in_=P[:])
                nc.sync.dma_start(out=dbg["b0"][:], in_=b0[:])
                nc.sync.dma_start(out=dbg["b1"][:], in_=b1[:])
                nc.sync.dma_start(out=dbg["m1"][:], in_=m1[:])
                nc.sync.dma_start(out=dbg["xpool"][:], in_=xpool[:])
                nc.sync.dma_start(out=dbg["x2T0"][:], in_=x2T[0][:])
                nc.sync.dma_start(out=dbg["x2T1"][:], in_=x2T[1][:])

    nc.finalize()
    return nc


def _get_program(reps=1):
    global _PROGRAM, _PROGRAM_REPS
    if _PROGRAM is None or _PROGRAM_REPS != reps:
        _PROGRAM = _build_program(reps)
        _PROGRAM_REPS = reps
    return _PROGRAM


def _prep_core(core, x, rows, cols, row1, col1, W1, W2, fc1_W, fc1_b, fc2_W, fc2_b):
    """Build the per-core input map (numpy only)."""
    g0 = GPC * core
    n0 = NPG * g0

    # x^T packed [128, 16000]: rows 0-63 = features x nodes [0,16000),
    # rows 64-127 = features x nodes [16000,32000) of this core's slice.
    xc = x[n0 : n0 + NPC]
    xt = np.empty((128, NPC // 2), np.float32)
    xt[0:64] = xc[: NPC // 2].T
    xt[64:128] = xc[NPC // 2 :].T

    eidx = np.zeros((128, EPG // 16), np.int16)
    bidx = np.zeros((128, 2 * BPAD // 16), np.int16)
    for g in range(GPC):
        gg = g0 + g
        e0 = EPG * gg
        r = rows[e0 : e0 + EPG] - NPG * gg
        c = cols[e0 : e0 + EPG] - NPG * gg
        order = np.argsort(r, kind="stable")
        cs = c[order].astype(np.int16)
        ends = np.cumsum(np.bincount(r, minlength=NPG))
        grp = _grp(g)
        for hh in range(2):
            seg = cs[HALF * hh : HALF * (hh + 1)]
            eidx[16 * grp : 16 * grp + 16, (HALF // 16) * hh : (HALF // 16) * (hh + 1)] = (
                seg.reshape(HALF // 16, 16).T
            )
            lo, hi = HALF * hh, HALF * (hh + 1)
            bvals = np.empty(NPG + 1, np.int64)
            bvals[0] = 0
            bvals[1:] = ends[:NPG]
            bv = np.clip(bvals, lo, hi) - lo
            bpad = np.zeros(BPAD, np.int16)
            bpad[: NPG + 1] = bv.astype(np.int16)
            bidx[16 * grp : 16 * grp + 16, (BPAD // 16) * hh : (BPAD // 16) * (hh + 1)] = (
                bpad.reshape(BPAD // 16, 16).T
            )

    a1t = np.zeros((100, GPC * 4 * CPG0), np.float32)
    for g in range(GPC):
        gg = g0 + g
        sel = (row1 // CPG0) == gg
        rl = (row1[sel] - CPG0 * gg).astype(np.int64)
        cl = (col1[sel] - CPG0 * gg).astype(np.int64)
        A1T = np.zeros((CPG0, CPG0), np.float32)
        A1T[cl, rl] = 1.0  # [src, dst]
        for ch in range(4):
            a1t[:, 4 * CPG0 * g + CPG0 * ch : 4 * CPG0 * g + CPG0 * (ch + 1)] = A1T[
                100 * ch : 100 * (ch + 1), :
            ]

    w1bd = np.zeros((128, 2 * F1), np.float32)
    w1bd[0:64, 0:F1] = W1
    w1bd[64:128, F1 : 2 * F1] = W1

    w2big = np.zeros((128, 4 * F2), np.float32)
    for q in range(2):
        for u in range(4):
            w2big[64 * q + 16 * u : 64 * q + 16 * (u + 1), F2 * u : F2 * (u + 1)] = W2

    fc1big = np.zeros((128, 128), np.float32)
    for q in range(2):
        for u in range(2):
            fc1big[64 * q + 32 * u : 64 * q + 32 * (u + 1), 64 * u : 64 * (u + 1)] = fc1_W

    return {
        "xt": xt,
        "eidx": eidx,
        "bidx": bidx,
        "a1t": a1t,
        "w1bd": w1bd,
        "w2big": w2big,
        "fc1big": fc1big,
        "fc1b": fc1_b.reshape(64, 1).astype(np.float32),
        "fc2w": fc2_W.reshape(64, 1).astype(np.float32),
        "fc2b": fc2_b.reshape(1, 1).astype(np.float32),
    }


def prepare_in_maps(**inputs):
    x = np.asarray(inputs["x"], np.float32)
    ei = np.asarray(inputs["edge_index"])
    rows, cols = np.asarray(ei[0]), np.asarray(ei[1])
    ei1 = np.asarray(inputs["edge_index1"])
    row1, col1 = np.asarray(ei1[0]), np.asarray(ei1[1])
    W1 = np.asarray(inputs["W1"], np.float32)
    W2 = np.asarray(inputs["W2"], np.float32)
    fc1_W = np.asarray(inputs["fc1_W"], np.float32)
    fc1_b = np.asarray(inputs["fc1_b"], np.float32)
    fc2_W = np.asarray(inputs["fc2_W"], np.float32)
    fc2_b = np.asarray(inputs["fc2_b"], np.float32)

    # structural assumptions baked into the kernel
    c0 = np.asarray(inputs["cluster0"])
    assert c0[0] == 0 and c0[-1] == B * CPG0 - 1
    assert np.array_equal(c0[::10], np.arange(B * CPG0))
    c1 = np.asarray(inputs["cluster1"])
    assert np.array_equal(c1[::10], np.arange(B * CPG1))
    b2 = np.asarray(inputs["batch2"])
    assert np.array_equal(b2[::CPG1], np.arange(B))

    return [
        _prep_core(c, x, rows, cols, row1, col1, W1, W2, fc1_W, fc1_b, fc2_W, fc2_b)
        for c in range(NCORES)
    ]


def run(in_maps, trace=False):
    from concourse.bass_utils import run_bass_kernel_spmd

    nc = _get_program()
    res = run_bass_kernel_spmd(nc, in_maps, list(range(NCORES)), trace=trace)
    out = np.empty((B, 1), np.float32)
    for c in range(NCORES):
        out[GPC * c : GPC * (c + 1), 0] = np.asarray(res.results[c]["out"][0])
    return out, res


def kernel(**inputs) -> np.ndarray:
    in_maps = prepare_in_maps(**inputs)
    out, _ = run(in_maps)
    return out



# revision 6
# speedup vs baseline: 1.0329x; 1.0329x over previous
"""Trainium2 Bass kernel for the hierarchical GNN (GINConv x2 + community pooling).

Math notes (vs the PyTorch/JAX reference):
  - softmax(alpha, axis=1) of an (E,1) tensor is identically 1, so the
    attention path collapses: conv(x) = segment_sum(xp[col], row) with
    xp = x @ W.  edge_attr / We / Wa are dead.
  - cluster0[i] == i//10, cluster1[i] == i//10, batch2[i] == i//40 (verified
    at runtime): the segment_max pools are max over blocks of 10 consecutive
    rows, and the per-graph mean divides by exactly 40.

Per-core layout (8 graphs per NeuronCore, data parallel over 8 cores):
  - conv1 runs in transposed layout: a [128, 4096] SBUF table holds xp^T for
    8 graphs (16 features x 8 graph-groups of 16 partitions).  Edge messages
    are gathered with GPSIMD ap_gather (per-16-partition-group indices = one
    graph per group), summed per destination via a chained DVE
    tensor_tensor_scan (prefix sum over dest-sorted edges) and a second
    ap_gather at segment boundaries + diff.
  - community pooling = strided max-reduce over blocks of 10 columns.
  - conv2 uses a dense per-graph 400x400 pooled adjacency (host-built from
    edge_index1) on the TensorEngine.
"""

import sys

sys.path.insert(0, "/opt/trn_rl_repo")

import numpy as np

B = 64
NPG = 4000
CPG0 = 400
CPG1 = 40
DEG = 8
F_IN = 64
F1 = 16
F2 = 32
EPG = NPG * DEG          # 32000 edges per graph
NCORES = 8
GPC = B // NCORES        # 8 graphs per core
NPC = GPC * NPG          # 32000 nodes per core
HALF = EPG // 2          # 16000 edges per scan half
CHUNK = 2000             # edges per gather/scan chunk
NCH = HALF // CHUNK      # 8 chunks per half
TBL = 4096               # xp^T table width (>= NPG)
BPAD = 4016              # boundary gather count (4001 rounded to x16)
XTC = 2000               # x^T DMA chunk columns
PJ = 500                 # projection matmul free-dim


def _grp(g):
    """Local graph id -> 16-partition table group (pairing (p, p+4))."""
    return 2 * g if g < 4 else 2 * (g - 4) + 1


_PROGRAM = None
_PROGRAM_REPS = None
_DEBUG = False


def _build_program(reps=1):
    import concourse.bacc as bacc
    import concourse.mybir as mybir
    from concourse.tile import TileContext

    f32 = mybir.dt.float32
    bf16 = mybir.dt.bfloat16
    i16 = mybir.dt.int16
    AF = mybir.ActivationFunctionType
    OP = mybir.AluOpType

    nc = bacc.Bacc(None, target_bir_lowering=False)

    xt_in = nc.declare_dram_parameter("xt", [128, NPC // 2], bf16, isOutput=False)
    eidx_in = nc.declare_dram_parameter("eidx", [128, EPG // 16], i16, isOutput=False)
    bidx_in = nc.declare_dram_parameter("bidx", [128, 2 * BPAD // 16], i16, isOutput=False)
    a1t_in = nc.declare_dram_parameter("a1t", [100, GPC * 4 * CPG0], bf16, isOutput=False)
    w1bd_in = nc.declare_dram_parameter("w1bd", [128, 2 * F1], bf16, isOutput=False)
    w2big_in = nc.declare_dram_parameter("w2big", [128, 4 * F2], bf16, isOutput=False)
    fc1big_in = nc.declare_dram_parameter("fc1big", [128, 128], f32, isOutput=False)
    fc1b_in = nc.declare_dram_parameter("fc1b", [64, 1], f32, isOutput=False)
    fc2w_in = nc.declare_dram_parameter("fc2w", [64, 1], f32, isOutput=False)
    fc2b_in = nc.declare_dram_parameter("fc2b", [1, 1], f32, isOutput=False)
    out_ext = nc.declare_dram_parameter("out", [1, GPC], f32, isOutput=True)
    dbg = {}
    if _DEBUG:
        dbg["xpT"] = nc.declare_dram_parameter("dbg_xpT", [128, TBL], f32, isOutput=True)
        dbg["P"] = nc.declare_dram_parameter("dbg_P", [128, HALF + 4], f32, isOutput=True)
        dbg["b0"] = nc.declare_dram_parameter("dbg_b0", [128, BPAD], f32, isOutput=True)
        dbg["b1"] = nc.declare_dram_parameter("dbg_b1", [128, BPAD], f32, isOutput=True)
        dbg["m1"] = nc.declare_dram_parameter("dbg_m1", [128, NPG], f32, isOutput=True)
        dbg["xpool"] = nc.declare_dram_parameter("dbg_xpool", [128, CPG0], f32, isOutput=True)
        dbg["x2T0"] = nc.declare_dram_parameter("dbg_x2T0", [128, CPG0], f32, isOutput=True)
        dbg["x2T1"] = nc.declare_dram_parameter("dbg_x2T1", [128, CPG0], f32, isOutput=True)

    with TileContext(nc) as tc:
        with (
            tc.tile_pool(name="sb", bufs=1) as sb,
            tc.tile_pool(name="xtp", bufs=2) as xtp,
            tc.tile_pool(name="gp", bufs=2) as gp,
            tc.tile_pool(name="ip", bufs=3) as ip,
            tc.tile_pool(name="a1p", bufs=8) as a1p,
            tc.tile_pool(name="ps", bufs=8, space="PSUM") as psp,
        ):
            w1bd = sb.tile([128, 2 * F1], bf16)
            w2big = sb.tile([128, 4 * F2], bf16)
            fc1big = sb.tile([128, 128], f32)
            fc1b = sb.tile([64, 1], f32)
            fc2w = sb.tile([64, 1], f32)
            fc2b = sb.tile([1, 1], f32)
            xpT = sb.tile([128, TBL], f32)
            P = sb.tile([128, HALF + 4], f32)
            b0 = sb.tile([128, BPAD], f32)
            b1 = sb.tile([128, BPAD], f32)
            xpool = sb.tile([128, CPG0], f32)
            xpoolb = sb.tile([128, CPG0], bf16)
            x2T = [sb.tile([128, CPG0], f32, name=f"x2T{s}") for s in range(2)]
            x3T = [sb.tile([128, CPG1], f32, name=f"x3T{s}") for s in range(2)]
            xg = [sb.tile([128, 1], f32, name=f"xg{s}") for s in range(2)]
            outt = sb.tile([1, GPC], f32)
            zero = sb.tile([128, 1], f32)

            for _rep in range(reps):
                nc.sync.dma_start(out=w1bd[:], in_=w1bd_in[:])
                nc.sync.dma_start(out=w2big[:], in_=w2big_in[:])
                nc.sync.dma_start(out=fc1big[:], in_=fc1big_in[:])
                nc.sync.dma_start(out=fc1b[:], in_=fc1b_in[:])
                nc.sync.dma_start(out=fc2w[:], in_=fc2w_in[:])
                nc.sync.dma_start(out=fc2b[:], in_=fc2b_in[:])
                nc.vector.memset(zero[:], 0.0)
                nc.vector.memset(P[:, 0:1], 0.0)

                # ---- stage 1: xp^T = (x @ W1)^T via paired block-diagonal W1 ----
                # xt chunk j covers x^T columns [XTC*j, XTC*(j+1)) of the packed
                # [128, 16000] layout; graph pair p = chunk column / NPG.
                for j in range(NPC // 2 // XTC):
                    xtt = xtp.tile([128, XTC], bf16, name="xtt")
                    nc.sync.dma_start(out=xtt[:], in_=xt_in[:, XTC * j : XTC * (j + 1)])
                    p = (XTC * j) // NPG
                    for i in range(XTC // PJ):
                        col = XTC * j + PJ * i - NPG * p  # table column base
                        ps = psp.tile([F2, PJ], f32, name="pj", tag="ps")
                        nc.tensor.matmul(
                            ps[:], w1bd[:], xtt[:, PJ * i : PJ * (i + 1)],
                            start=True, stop=True,
                        )
                        nc.scalar.activation(
                            xpT[32 * p : 32 * p + 32, col : col + PJ], ps[:], AF.Copy
                        )
                # prefetch all per-graph pooled adjacencies (bf16) during the
                # gather phase; sync-queue FIFO puts these after the xt chunks.
                a1_tiles = []
                for g in range(GPC):
                    a1 = a1p.tile([100, 4, CPG0], bf16, name="a1g")
                    nc.sync.dma_start(
                        out=a1[:], in_=a1t_in[:, 4 * CPG0 * g : 4 * CPG0 * (g + 1)]
                    )
                    a1_tiles.append(a1)

                # ---- stage 2: conv1 aggregation (gather + prefix scan + diff) ----
                zbc = zero[:, 0:1].broadcast_to((128, CHUNK))
                for hh, bt in ((0, b0), (1, b1)):
                    for k in range(NCH):
                        g = gp.tile([128, CHUNK], f32, name="gch")
                        ic = (HALF // 16) * hh + (CHUNK // 16) * k
                        # ap_gather ucode ignores the idx AP offset: stage each
                        # chunk's indices into an offset-0 tile via DMA.
                        et = ip.tile([128, CHUNK // 16], i16, name="et")
                        nc.sync.dma_start(out=et[:], in_=eidx_in[:, ic : ic + CHUNK // 16])
                        nc.gpsimd.ap_gather(
                            g[:], xpT[:], et[:],
                            channels=128, num_elems=TBL, d=1, num_idxs=CHUNK,
                        )
                        init = 0.0 if k == 0 else P[:, CHUNK * k : CHUNK * k + 1]
                        nc.vector.tensor_tensor_scan(
                            P[:, 1 + CHUNK * k : 1 + CHUNK * (k + 1)],
                            g[:], zbc, init, OP.add, OP.add,
                        )
                    bix = ip.tile([128, BPAD // 16], i16, name="bix", bufs=2)
                    nc.sync.dma_start(
                        out=bix[:],
                        in_=bidx_in[:, (BPAD // 16) * hh : (BPAD // 16) * (hh + 1)],
                    )
                    nc.gpsimd.ap_gather(
                        bt[:], P[:, 0 : HALF + 1], bix[:],
                        channels=128, num_elems=HALF + 1, d=1, num_idxs=BPAD,
                    )
                nc.vector.tensor_tensor(
                    b0[:, 0 : NPG + 1], b0[:, 0 : NPG + 1], b1[:, 0 : NPG + 1], OP.add
                )
                # m1 aliases b1's storage (b1 is dead after the add above)
                m1 = b1[:, 0:NPG]
                nc.vector.tensor_tensor(
                    m1, b0[:, 1 : NPG + 1], b0[:, 0:NPG], OP.subtract
                )
                # pool1: relu(max over blocks of 10) == max(relu) since relu monotone
                nc.vector.tensor_reduce(
                    xpool[:], m1.rearrange("p (a b) -> p a b", b=10),
                    mybir.AxisListType.X, OP.max,
                )
                nc.scalar.activation(xpoolb[:], xpool[:], AF.Relu)

                # ---- stage 3: conv2 = relu(A1 @ (x_pool @ W2)) per graph ----
                # phase 1 for all graphs first, then phase 2, so independent
                # per-graph chains pipeline across engines.
                xp2t_tiles = []
                for g in range(GPC):
                    t = _grp(g)
                    q, u = t // 4, t % 4
                    xp2t = sb.tile([100, 4, F2], bf16, name="xp2t", bufs=8)
                    for ch in range(4):
                        ps = psp.tile([100, F2], f32, name="p2", tag="ps")
                        nc.tensor.matmul(
                            ps[:],
                            xpoolb[64 * q : 64 * (q + 1), 100 * ch : 100 * (ch + 1)],
                            w2big[64 * q : 64 * (q + 1), F2 * u : F2 * (u + 1)],
                            start=True, stop=True,
                        )
                        nc.scalar.activation(xp2t[:, ch, :], ps[:], AF.Copy)
                    xp2t_tiles.append(xp2t)
                for g in range(GPC):
                    psm = psp.tile([F2, CPG0], f32, name="pm", tag="ps")
                    for ch in range(4):
                        nc.tensor.matmul(
                            psm[:], xp2t_tiles[g][:, ch, :], a1_tiles[g][:, ch, :],
                            start=(ch == 0), stop=(ch == 3),
                        )
                    s, slot = g // 4, g % 4
                    nc.scalar.activation(
                        x2T[s][32 * slot : 32 * (slot + 1), :], psm[:], AF.Relu
                    )

                # ---- stage 4: pool2 + graph mean + MLP ----
                for s in range(2):
                    nc.vector.tensor_reduce(
                        x3T[s][:], x2T[s][:].rearrange("p (a b) -> p a b", b=10),
                        mybir.AxisListType.X, OP.max,
                    )
                    nc.vector.tensor_reduce(
                        xg[s][:], x3T[s][:], mybir.AxisListType.X, OP.add
                    )
                for g in range(GPC):
                    s, slot = g // 4, g % 4
                    q2, u2 = slot // 2, slot % 2
                    psh = psp.tile([64, 1], f32, name="ph", tag="ps")
                    nc.tensor.matmul(
                        psh[:],
                        fc1big[64 * q2 : 64 * (q2 + 1), 64 * u2 : 64 * (u2 + 1)],
                        xg[s][64 * q2 : 64 * (q2 + 1), 0:1],
                        start=True, stop=True,
                    )
                    ht = sb.tile([64, 1], f32, name="ht", bufs=2)
                    nc.scalar.activation(
                        ht[:], psh[:], AF.Relu, bias=fc1b[:], scale=1.0 / CPG1
                    )
                    pso = psp.tile([1, 1], f32, name="po", tag="ps")
                    nc.tensor.matmul(pso[:], fc2w[:], ht[:], start=True, stop=True)
                    # final linear has no relu: out = h @ fc2_W + fc2_b
                    nc.vector.scalar_tensor_tensor(
                        outt[0:1, g : g + 1], pso[:], 1.0, fc2b[0:1, 0:1],
                        OP.mult, OP.add,
                    )
                nc.sync.dma_start(out=out_ext[:], in_=outt[:])
            if _DEBUG:
                nc.sync.dma_start(out=dbg["xpT"][:], in_=xpT[:])
                nc.sync.dma_start(out=dbg["P"][:], in_=P[:])
                nc.sync.dma_start(out=dbg["b0"][:], in_=b0[:])
                nc.sync.dma_start(out=dbg["b1"][:], in_=b1[:])
                nc.sync.dma_start(out=dbg["m1"][:], in_=m1[:])
                nc.sync.dma_start(out=dbg["xpool"][:], in_=xpool[:])
                nc.sync.dma_start(out=dbg["x2T0"][:], in_=x2T[0][:])
                nc.sync.dma_start(out=dbg["x2T1"][:], in_=x2T[1][:])

    nc.finalize()
    return nc


def _get_program(reps=1):
    global _PROGRAM, _PROGRAM_REPS
    if _PROGRAM is None or _PROGRAM_REPS != reps:
        _PROGRAM = _build_program(reps)
        _PROGRAM_REPS = reps
    return _PROGRAM


def _prep_core(core, x, rows, cols, row1, col1, W1, W2, fc1_W, fc1_b, fc2_W, fc2_b):
    """Build the per-core input map (numpy only)."""
    import ml_dtypes

    bf16 = ml_dtypes.bfloat16
    g0 = GPC * core
    n0 = NPG * g0

    # x^T packed [128, 16000]: rows 0-63 = features x nodes [0,16000),
    # rows 64-127 = features x nodes [16000,32000) of this core's slice.
    xc = x[n0 : n0 + NPC]
    xt = np.empty((128, NPC // 2), bf16)
    xt[0:64] = xc[: NPC // 2].T
    xt[64:128] = xc[NPC // 2 :].T

    eidx = np.zeros((128, EPG // 16), np.int16)
    bidx = np.zeros((128, 2 * BPAD // 16), np.int16)
    for g in range(GPC):
        gg = g0 + g
        e0 = EPG * gg
        r = rows[e0 : e0 + EPG] - NPG * gg
        c = cols[e0 : e0 + EPG] - NPG * gg
        order = np.argsort(r, kind="stable")
        cs = c[order].astype(np.int16)
        ends = np.cumsum(np.bincount(r, minlength=NPG))
        grp = _grp(g)
        for hh in range(2):
            seg = cs[HALF * hh : HALF * (hh + 1)]
            eidx[16 * grp : 16 * grp + 16, (HALF // 16) * hh : (HALF // 16) * (hh + 1)] = (
                seg.reshape(HALF // 16, 16).T
            )
            lo, hi = HALF * hh, HALF * (hh + 1)
            bvals = np.empty(NPG + 1, np.int64)
            bvals[0] = 0
            bvals[1:] = ends[:NPG]
            bv = np.clip(bvals, lo, hi) - lo
            bpad = np.zeros(BPAD, np.int16)
            bpad[: NPG + 1] = bv.astype(np.int16)
            bidx[16 * grp : 16 * grp + 16, (BPAD // 16) * hh : (BPAD // 16) * (hh + 1)] = (
                bpad.reshape(BPAD // 16, 16).T
            )

    a1t = np.zeros((100, GPC * 4 * CPG0), bf16)
    for g in range(GPC):
        gg = g0 + g
        sel = (row1 // CPG0) == gg
        rl = (row1[sel] - CPG0 * gg).astype(np.int64)
        cl = (col1[sel] - CPG0 * gg).astype(np.int64)
        A1T = np.zeros((CPG0, CPG0), np.float32)
        A1T[cl, rl] = 1.0  # [src, dst]
        for ch in range(4):
            a1t[:, 4 * CPG0 * g + CPG0 * ch : 4 * CPG0 * g + CPG0 * (ch + 1)] = A1T[
                100 * ch : 100 * (ch + 1), :
            ]

    w1bd = np.zeros((128, 2 * F1), bf16)
    w1bd[0:64, 0:F1] = W1
    w1bd[64:128, F1 : 2 * F1] = W1

    w2big = np.zeros((128, 4 * F2), bf16)
    for q in range(2):
        for u in range(4):
            w2big[64 * q + 16 * u : 64 * q + 16 * (u + 1), F2 * u : F2 * (u + 1)] = W2

    fc1big = np.zeros((128, 128), np.float32)
    for q in range(2):
        for u in range(2):
            fc1big[64 * q + 32 * u : 64 * q + 32 * (u + 1), 64 * u : 64 * (u + 1)] = fc1_W

    return {
        "xt": xt,
        "eidx": eidx,
        "bidx": bidx,
        "a1t": a1t,
        "w1bd": w1bd,
        "w2big": w2big,
        "fc1big": fc1big,
        "fc1b": fc1_b.reshape(64, 1).astype(np.float32),
        "fc2w": fc2_W.reshape(64, 1).astype(np.float32),
        "fc2b": fc2_b.reshape(1, 1).astype(np.float32),
    }


def prepare_in_maps(**inputs):
    x = np.asarray(inputs["x"], np.float32)
    ei = np.asarray(inputs["edge_index"])
    rows, cols = np.asarray(ei[0]), np.asarray(ei[1])
    ei1 = np.asarray(inputs["edge_index1"])
    row1, col1 = np.asarray(ei1[0]), np.asarray(ei1[1])
    W1 = np.asarray(inputs["W1"], np.float32)
    W2 = np.asarray(inputs["W2"], np.float32)
    fc1_W = np.asarray(inputs["fc1_W"], np.float32)
    fc1_b = np.asarray(inputs["fc1_b"], np.float32)
    fc2_W = np.asarray(inputs["fc2_W"], np.float32)
    fc2_b = np.asarray(inputs["fc2_b"], np.float32)

    # structural assumptions baked into the kernel
    c0 = np.asarray(inputs["cluster0"])
    assert c0[0] == 0 and c0[-1] == B * CPG0 - 1
    assert np.array_equal(c0[::10], np.arange(B * CPG0))
    c1 = np.asarray(inputs["cluster1"])
    assert np.array_equal(c1[::10], np.arange(B * CPG1))
    b2 = np.asarray(inputs["batch2"])
    assert np.array_equal(b2[::CPG1], np.arange(B))

    return [
        _prep_core(c, x, rows, cols, row1, col1, W1, W2, fc1_W, fc1_b, fc2_W, fc2_b)
        for c in range(NCORES)
    ]


def run(in_maps, trace=False):
    from concourse.bass_utils import run_bass_kernel_spmd

    nc = _get_program()
    res = run_bass_kernel_spmd(nc, in_maps, list(range(NCORES)), trace=trace)
    out = np.empty((B, 1), np.float32)
    for c in range(NCORES):
        out[GPC * c : GPC * (c + 1), 0] = np.asarray(res.results[c]["out"][0])
    return out, res


def kernel(**inputs) -> np.ndarray:
    in_maps = prepare_in_maps(**inputs)
    out, _ = run(in_maps)
    return out



# revision 9
# speedup vs baseline: 1.0435x; 1.0103x over previous
"""Trainium2 Bass kernel for the hierarchical GNN (GINConv x2 + community pooling).

Math notes (vs the PyTorch/JAX reference):
  - softmax(alpha, axis=1) of an (E,1) tensor is identically 1, so the
    attention path collapses: conv(x) = segment_sum(xp[col], row) with
    xp = x @ W.  edge_attr / We / Wa are dead.
  - cluster0[i] == i//10, cluster1[i] == i//10, batch2[i] == i//40 (verified
    at runtime): the segment_max pools are max over blocks of 10 consecutive
    rows, and the per-graph mean divides by exactly 40.

Per-core layout (8 graphs per NeuronCore, data parallel over 8 cores):
  - conv1 runs in transposed layout: a [128, 4096] SBUF table holds xp^T for
    8 graphs (16 features x 8 graph-groups of 16 partitions).  Edge messages
    are gathered with GPSIMD ap_gather (per-16-partition-group indices = one
    graph per group), summed per destination via a chained DVE
    tensor_tensor_scan (prefix sum over dest-sorted edges) and a second
    ap_gather at segment boundaries + diff.
  - community pooling = strided max-reduce over blocks of 10 columns.
  - conv2 uses a dense per-graph 400x400 pooled adjacency (host-built from
    edge_index1) on the TensorEngine.
"""

import sys

sys.path.insert(0, "/opt/trn_rl_repo")

import numpy as np

B = 64
NPG = 4000
CPG0 = 400
CPG1 = 40
DEG = 8
F_IN = 64
F1 = 16
F2 = 32
EPG = NPG * DEG          # 32000 edges per graph
NCORES = 8
GPC = B // NCORES        # 8 graphs per core
NPC = GPC * NPG          # 32000 nodes per core
HALF = EPG // 2          # 16000 edges per scan half
CHUNK = 2000             # edges per gather/scan chunk
NCH = HALF // CHUNK      # 8 chunks per half
TBL = 4096               # xp^T table width (>= NPG)
BPAD = 4016              # boundary gather count (4001 rounded to x16)
XTC = 2000               # x^T DMA chunk columns
PJ = 500                 # projection matmul free-dim


def _grp(g):
    """Local graph id -> 16-partition table group (pairing (p, p+4))."""
    return 2 * g if g < 4 else 2 * (g - 4) + 1


_PROGRAM = None
_PROGRAM_REPS = None
_DEBUG = False


def _build_program(reps=1):
    import concourse.bacc as bacc
    import concourse.mybir as mybir
    from concourse.tile import TileContext

    f32 = mybir.dt.float32
    bf16 = mybir.dt.bfloat16
    i16 = mybir.dt.int16
    AF = mybir.ActivationFunctionType
    OP = mybir.AluOpType

    nc = bacc.Bacc(None, target_bir_lowering=False)

    xt_in = nc.declare_dram_parameter("xt", [128, NPC // 2], bf16, isOutput=False)
    eidx_in = nc.declare_dram_parameter("eidx", [128, EPG // 16], i16, isOutput=False)
    bidx_in = nc.declare_dram_parameter("bidx", [128, 2 * BPAD // 16], i16, isOutput=False)
    a1t_in = nc.declare_dram_parameter("a1t", [100, GPC * 4 * CPG0], bf16, isOutput=False)
    w1bd_in = nc.declare_dram_parameter("w1bd", [128, 2 * F1], bf16, isOutput=False)
    w2big_in = nc.declare_dram_parameter("w2big", [128, 4 * F2], bf16, isOutput=False)
    fc1big_in = nc.declare_dram_parameter("fc1big", [128, 128], f32, isOutput=False)
    fc1b_in = nc.declare_dram_parameter("fc1b", [64, 1], f32, isOutput=False)
    fc2w_in = nc.declare_dram_parameter("fc2w", [64, 1], f32, isOutput=False)
    fc2b_in = nc.declare_dram_parameter("fc2b", [1, 1], f32, isOutput=False)
    out_ext = nc.declare_dram_parameter("out", [1, GPC], f32, isOutput=True)
    dbg = {}
    if _DEBUG:
        dbg["xpT"] = nc.declare_dram_parameter("dbg_xpT", [128, TBL], f32, isOutput=True)
        dbg["P"] = nc.declare_dram_parameter("dbg_P", [128, HALF + 4], f32, isOutput=True)
        dbg["b0"] = nc.declare_dram_parameter("dbg_b0", [128, BPAD], f32, isOutput=True)
        dbg["b1"] = nc.declare_dram_parameter("dbg_b1", [128, BPAD], f32, isOutput=True)
        dbg["m1"] = nc.declare_dram_parameter("dbg_m1", [128, NPG], f32, isOutput=True)
        dbg["xpool"] = nc.declare_dram_parameter("dbg_xpool", [128, CPG0], f32, isOutput=True)
        dbg["x2T0"] = nc.declare_dram_parameter("dbg_x2T0", [128, CPG0], f32, isOutput=True)
        dbg["x2T1"] = nc.declare_dram_parameter("dbg_x2T1", [128, CPG0], f32, isOutput=True)

    with TileContext(nc) as tc:
        with (
            tc.tile_pool(name="sb", bufs=1) as sb,
            tc.tile_pool(name="xtp", bufs=2) as xtp,
            tc.tile_pool(name="gp", bufs=2) as gp,
            tc.tile_pool(name="ip", bufs=3) as ip,
            tc.tile_pool(name="a1p", bufs=8) as a1p,
            tc.tile_pool(name="ps", bufs=8, space="PSUM") as psp,
        ):
            w1bd = sb.tile([128, 2 * F1], bf16)
            w2big = sb.tile([128, 4 * F2], bf16)
            fc1big = sb.tile([128, 128], f32)
            fc1b = sb.tile([64, 1], f32)
            fc2w = sb.tile([64, 1], f32)
            fc2b = sb.tile([1, 1], f32)
            xpT = sb.tile([128, TBL], f32)
            P = sb.tile([128, HALF + 4], f32)
            b0 = sb.tile([128, BPAD], f32)
            b1 = sb.tile([128, BPAD], f32)
            xpool = sb.tile([128, CPG0], f32)
            xpoolb = sb.tile([128, CPG0], bf16)
            x2T = [sb.tile([128, CPG0], f32, name=f"x2T{s}") for s in range(2)]
            x3T = [sb.tile([128, CPG1], f32, name=f"x3T{s}") for s in range(2)]
            xg = [sb.tile([128, 1], f32, name=f"xg{s}") for s in range(2)]
            outt = sb.tile([1, GPC], f32)
            zero = sb.tile([128, 1], f32)

            for _rep in range(reps):
                nc.sync.dma_start(out=w1bd[:], in_=w1bd_in[:])
                nc.sync.dma_start(out=w2big[:], in_=w2big_in[:])
                nc.sync.dma_start(out=fc1big[:], in_=fc1big_in[:])
                nc.sync.dma_start(out=fc1b[:], in_=fc1b_in[:])
                nc.sync.dma_start(out=fc2w[:], in_=fc2w_in[:])
                nc.sync.dma_start(out=fc2b[:], in_=fc2b_in[:])
                nc.vector.memset(zero[:], 0.0)
                nc.vector.memset(P[:, 0:1], 0.0)

                # ---- stage 1: xp^T = (x @ W1)^T via paired block-diagonal W1 ----
                # xt chunk j covers x^T columns [XTC*j, XTC*(j+1)) of the packed
                # [128, 16000] layout; graph pair p = chunk column / NPG.
                for j in range(NPC // 2 // XTC):
                    xtt = xtp.tile([128, XTC], bf16, name="xtt")
                    nc.sync.dma_start(out=xtt[:], in_=xt_in[:, XTC * j : XTC * (j + 1)])
                    p = (XTC * j) // NPG
                    for i in range(XTC // PJ):
                        col = XTC * j + PJ * i - NPG * p  # table column base
                        ps = psp.tile([F2, PJ], f32, name="pj", tag="ps")
                        nc.tensor.matmul(
                            ps[:], w1bd[:], xtt[:, PJ * i : PJ * (i + 1)],
                            start=True, stop=True,
                        )
                        nc.scalar.activation(
                            xpT[32 * p : 32 * p + 32, col : col + PJ], ps[:], AF.Copy
                        )
                # ---- stage 2: conv1 aggregation (gather + prefix scan + diff) ----
                a1_tiles = []
                zbc = zero[:, 0:1].broadcast_to((128, CHUNK))
                for hh, bt in ((0, b0), (1, b1)):
                    for k in range(NCH):
                        g = gp.tile([128, CHUNK], f32, name="gch")
                        ic = (HALF // 16) * hh + (CHUNK // 16) * k
                        # ap_gather ucode ignores the idx AP offset: stage each
                        # chunk's indices into an offset-0 tile via DMA.
                        et = ip.tile([128, CHUNK // 16], i16, name="et")
                        nc.sync.dma_start(out=et[:], in_=eidx_in[:, ic : ic + CHUNK // 16])
                        nc.gpsimd.ap_gather(
                            g[:], xpT[:], et[:],
                            channels=128, num_elems=TBL, d=1, num_idxs=CHUNK,
                        )
                        init = 0.0 if k == 0 else P[:, CHUNK * k : CHUNK * k + 1]
                        nc.vector.tensor_tensor_scan(
                            P[:, 1 + CHUNK * k : 1 + CHUNK * (k + 1)],
                            g[:], zbc, init, OP.add, OP.add,
                        )
                    if hh == 0:
                        # prefetch per-graph pooled adjacencies (bf16) on the
                        # sync queue now that half-0 eidx staging is queued.
                        for g in range(GPC):
                            a1 = a1p.tile([100, 4, CPG0], bf16, name="a1g")
                            nc.sync.dma_start(
                                out=a1[:],
                                in_=a1t_in[:, 4 * CPG0 * g : 4 * CPG0 * (g + 1)],
                            )
                            a1_tiles.append(a1)
                    bix = ip.tile([128, BPAD // 16], i16, name="bix", bufs=2)
                    nc.sync.dma_start(
                        out=bix[:],
                        in_=bidx_in[:, (BPAD // 16) * hh : (BPAD // 16) * (hh + 1)],
                    )
                    nc.gpsimd.ap_gather(
                        bt[:], P[:, 0 : HALF + 1], bix[:],
                        channels=128, num_elems=HALF + 1, d=1, num_idxs=BPAD,
                    )
                nc.vector.tensor_tensor(
                    b0[:, 0 : NPG + 1], b0[:, 0 : NPG + 1], b1[:, 0 : NPG + 1], OP.add
                )
                # m1 aliases b1's storage (b1 is dead after the add above)
                m1 = b1[:, 0:NPG]
                nc.vector.tensor_tensor(
                    m1, b0[:, 1 : NPG + 1], b0[:, 0:NPG], OP.subtract
                )
                # pool1: relu(max over blocks of 10) == max(relu) since relu monotone
                nc.vector.tensor_reduce(
                    xpool[:], m1.rearrange("p (a b) -> p a b", b=10),
                    mybir.AxisListType.X, OP.max,
                )
                nc.scalar.activation(xpoolb[:], xpool[:], AF.Relu)

                # ---- stage 3: conv2 = relu(A1 @ (x_pool @ W2)) per graph ----
                # phase 1 for all graphs first, then phase 2, so independent
                # per-graph chains pipeline across engines.
                xp2t_tiles = []
                for g in range(GPC):
                    t = _grp(g)
                    q, u = t // 4, t % 4
                    xp2t = sb.tile([100, 4, F2], bf16, name="xp2t", bufs=8)
                    for ch in range(4):
                        ps = psp.tile([100, F2], f32, name="p2", tag="ps")
                        nc.tensor.matmul(
                            ps[:],
                            xpoolb[64 * q : 64 * (q + 1), 100 * ch : 100 * (ch + 1)],
                            w2big[64 * q : 64 * (q + 1), F2 * u : F2 * (u + 1)],
                            start=True, stop=True,
                        )
                        nc.scalar.activation(xp2t[:, ch, :], ps[:], AF.Copy)
                    xp2t_tiles.append(xp2t)
                for g in range(GPC):
                    psm = psp.tile([F2, CPG0], f32, name="pm", tag="ps")
                    for ch in range(4):
                        nc.tensor.matmul(
                            psm[:], xp2t_tiles[g][:, ch, :], a1_tiles[g][:, ch, :],
                            start=(ch == 0), stop=(ch == 3),
                        )
                    s, slot = g // 4, g % 4
                    nc.scalar.activation(
                        x2T[s][32 * slot : 32 * (slot + 1), :], psm[:], AF.Relu
                    )

                # ---- stage 4: pool2 + graph mean + MLP ----
                for s in range(2):
                    nc.vector.tensor_reduce(
                        x3T[s][:], x2T[s][:].rearrange("p (a b) -> p a b", b=10),
                        mybir.AxisListType.X, OP.max,
                    )
                    nc.vector.tensor_reduce(
                        xg[s][:], x3T[s][:], mybir.AxisListType.X, OP.add
                    )
                for g in range(GPC):
                    s, slot = g // 4, g % 4
                    q2, u2 = slot // 2, slot % 2
                    psh = psp.tile([64, 1], f32, name="ph", tag="ps")
                    nc.tensor.matmul(
                        psh[:],
                        fc1big[64 * q2 : 64 * (q2 + 1), 64 * u2 : 64 * (u2 + 1)],
                        xg[s][64 * q2 : 64 * (q2 + 1), 0:1],
                        start=True, stop=True,
                    )
                    ht = sb.tile([64, 1], f32, name="ht", bufs=2)
                    nc.scalar.activation(
                        ht[:], psh[:], AF.Relu, bias=fc1b[:], scale=1.0 / CPG1
                    )
                    pso = psp.tile([1, 1], f32, name="po", tag="ps")
                    nc.tensor.matmul(pso[:], fc2w[:], ht[:], start=True, stop=True)
                    # final linear has no relu: out = h @ fc2_W + fc2_b
                    nc.vector.scalar_tensor_tensor(
                        outt[0:1, g : g + 1], pso[:], 1.0, fc2b[0:1, 0:1],
                        OP.mult, OP.add,
                    )
                nc.sync.dma_start(out=out_ext[:], in_=outt[:])
            if _DEBUG:
                nc.sync.dma_start(out=dbg["xpT"][:], in_=xpT[:])
                nc.sync.dma_start(out=dbg["P"][:], in_=P[:])
                nc.sync.dma_start(out=dbg["b0"][:], in_=b0[:])
                nc.sync.dma_start(out=dbg["b1"][:], in_=b1[:])
                nc.sync.dma_start(out=dbg["m1"][:], in_=m1[:])
                nc.sync.dma_start(out=dbg["xpool"][:], in_=xpool[:])
                nc.sync.dma_start(out=dbg["x2T0"][:], in_=x2T[0][:])
                nc.sync.dma_start(out=dbg["x2T1"][:], in_=x2T[1][:])

    nc.finalize()
    return nc


def _get_program(reps=1):
    global _PROGRAM, _PROGRAM_REPS
    if _PROGRAM is None or _PROGRAM_REPS != reps:
        _PROGRAM = _build_program(reps)
        _PROGRAM_REPS = reps
    return _PROGRAM


def _prep_core(core, x, rows, cols, row1, col1, W1, W2, fc1_W, fc1_b, fc2_W, fc2_b):
    """Build the per-core input map (numpy only)."""
    import ml_dtypes

    bf16 = ml_dtypes.bfloat16
    g0 = GPC * core
    n0 = NPG * g0

    # x^T packed [128, 16000]: rows 0-63 = features x nodes [0,16000),
    # rows 64-127 = features x nodes [16000,32000) of this core's slice.
    xc = x[n0 : n0 + NPC]
    xt = np.empty((128, NPC // 2), bf16)
    xt[0:64] = xc[: NPC // 2].T
    xt[64:128] = xc[NPC // 2 :].T

    eidx = np.zeros((128, EPG // 16), np.int16)
    bidx = np.zeros((128, 2 * BPAD // 16), np.int16)
    for g in range(GPC):
        gg = g0 + g
        e0 = EPG * gg
        r = rows[e0 : e0 + EPG] - NPG * gg
        c = cols[e0 : e0 + EPG] - NPG * gg
        order = np.argsort(r, kind="stable")
        cs = c[order].astype(np.int16)
        ends = np.cumsum(np.bincount(r, minlength=NPG))
        grp = _grp(g)
        for hh in range(2):
            seg = cs[HALF * hh : HALF * (hh + 1)]
            eidx[16 * grp : 16 * grp + 16, (HALF // 16) * hh : (HALF // 16) * (hh + 1)] = (
                seg.reshape(HALF // 16, 16).T
            )
            lo, hi = HALF * hh, HALF * (hh + 1)
            bvals = np.empty(NPG + 1, np.int64)
            bvals[0] = 0
            bvals[1:] = ends[:NPG]
            bv = np.clip(bvals, lo, hi) - lo
            bpad = np.zeros(BPAD, np.int16)
            bpad[: NPG + 1] = bv.astype(np.int16)
            bidx[16 * grp : 16 * grp + 16, (BPAD // 16) * hh : (BPAD // 16) * (hh + 1)] = (
                bpad.reshape(BPAD // 16, 16).T
            )

    a1t = np.zeros((100, GPC * 4 * CPG0), bf16)
    for g in range(GPC):
        gg = g0 + g
        sel = (row1 // CPG0) == gg
        rl = (row1[sel] - CPG0 * gg).astype(np.int64)
        cl = (col1[sel] - CPG0 * gg).astype(np.int64)
        A1T = np.zeros((CPG0, CPG0), np.float32)
        A1T[cl, rl] = 1.0  # [src, dst]
        for ch in range(4):
            a1t[:, 4 * CPG0 * g + CPG0 * ch : 4 * CPG0 * g + CPG0 * (ch + 1)] = A1T[
                100 * ch : 100 * (ch + 1), :
            ]

    w1bd = np.zeros((128, 2 * F1), bf16)
    w1bd[0:64, 0:F1] = W1
    w1bd[64:128, F1 : 2 * F1] = W1

    w2big = np.zeros((128, 4 * F2), bf16)
    for q in range(2):
        for u in range(4):
            w2big[64 * q + 16 * u : 64 * q + 16 * (u + 1), F2 * u : F2 * (u + 1)] = W2

    fc1big = np.zeros((128, 128), np.float32)
    for q in range(2):
        for u in range(2):
            fc1big[64 * q + 32 * u : 64 * q + 32 * (u + 1), 64 * u : 64 * (u + 1)] = fc1_W

    return {
        "xt": xt,
        "eidx": eidx,
        "bidx": bidx,
        "a1t": a1t,
        "w1bd": w1bd,
        "w2big": w2big,
        "fc1big": fc1big,
        "fc1b": fc1_b.reshape(64, 1).astype(np.float32),
        "fc2w": fc2_W.reshape(64, 1).astype(np.float32),
        "fc2b": fc2_b.reshape(1, 1).astype(np.float32),
    }


def prepare_in_maps(**inputs):
    x = np.asarray(inputs["x"], np.float32)
    ei = np.asarray(inputs["edge_index"])
    rows, cols = np.asarray(ei[0]), np.asarray(ei[1])
    ei1 = np.asarray(inputs["edge_index1"])
    row1, col1 = np.asarray(ei1[0]), np.asarray(ei1[1])
    W1 = np.asarray(inputs["W1"], np.float32)
    W2 = np.asarray(inputs["W2"], np.float32)
    fc1_W = np.asarray(inputs["fc1_W"], np.float32)
    fc1_b = np.asarray(inputs["fc1_b"], np.float32)
    fc2_W = np.asarray(inputs["fc2_W"], np.float32)
    fc2_b = np.asarray(inputs["fc2_b"], np.float32)

    # structural assumptions baked into the kernel
    c0 = np.asarray(inputs["cluster0"])
    assert c0[0] == 0 and c0[-1] == B * CPG0 - 1
    assert np.array_equal(c0[::10], np.arange(B * CPG0))
    c1 = np.asarray(inputs["cluster1"])
    assert np.array_equal(c1[::10], np.arange(B * CPG1))
    b2 = np.asarray(inputs["batch2"])
    assert np.array_equal(b2[::CPG1], np.arange(B))

    return [
        _prep_core(c, x, rows, cols, row1, col1, W1, W2, fc1_W, fc1_b, fc2_W, fc2_b)
        for c in range(NCORES)
    ]


def run(in_maps, trace=False):
    from concourse.bass_utils import run_bass_kernel_spmd

    nc = _get_program()
    res = run_bass_kernel_spmd(nc, in_maps, list(range(NCORES)), trace=trace)
    out = np.empty((B, 1), np.float32)
    for c in range(NCORES):
        out[GPC * c : GPC * (c + 1), 0] = np.asarray(res.results[c]["out"][0])
    return out, res


def kernel(**inputs) -> np.ndarray:
    in_maps = prepare_in_maps(**inputs)
    out, _ = run(in_maps)
    return out

